# revision 1
# baseline (speedup 1.0000x reference)
"""TP-8 Trainium2 Bass kernel for a LLaDA/Llama transformer block.

Shapes (hardcoded): x [2, 1024, 4096], 32 heads x 128 head_dim,
FF=12288, non-causal attention, RMSNorm + RoPE + SwiGLU.

Sharding (per sharding_hint): tensor-parallel over the 8 cores —
q/k/v/ff sharded on the output-feature axis (4 heads / 1536 ff dims per
core), wo/w_out sharded on the contraction axis.  One fp16 on-device
AllReduce (chunked) restores the residual stream after attention; the
final projection partials are summed on the host.

Device notes:
 - Activations live transposed: [D_on_partitions, tokens]; fp16
   operands everywhere on the PE (1 cyc/row, 11-bit mantissa).
 - rms_norm scale rs = exp(-0.5*ln(mean_sq + eps)) via two ACT ops;
   norm weights are folded into the adjacent weight matrices on the
   host; 1/sqrt(head_dim) is folded into wq.
 - Cross-partition sums (sum over D, softmax denominator) use an
   all-ones stationary operand on the tensor engine.
 - RoPE is applied during the PSUM eviction of the q/k projections.
 - Softmax needs no max subtraction (logits are O(5) here).
 - The MLP matmuls read the AllReduced fp16 stream directly; the norm2
   scale is folded into the ff/up PSUM evictions so only evictions are
   gated on the norm statistics.
"""

from contextlib import ExitStack

import numpy as np

import concourse.mybir as mybir
import concourse.tile as tile
from concourse import bacc
from concourse.bass_utils import run_bass_kernel_spmd

F32 = mybir.dt.float32
F16 = mybir.dt.float16
AF = mybir.ActivationFunctionType
ALU = mybir.AluOpType

N_CORES = 8
P = 128
B, T, D, FF = 2, 1024, 4096, 12288
M = B * T            # 2048 tokens
H = 128              # head dim
HALF = 64
QC = D // N_CORES    # 512 per-core q/k/v features (4 heads)
NH = QC // H         # 4 heads per core
FC = FF // N_CORES   # 1536 per-core ff features
NKP = D // P         # 32 K-tiles over D
NFT = FC // P        # 12 M-tiles over per-core FF
NDT = D // P         # 32 D-tiles
NST = T // P         # 8 sequence tiles per batch
EPS = 1e-05
AR_CHUNKS = 4


def _build():
    nc = bacc.Bacc("TRN2", target_bir_lowering=False, num_devices=N_CORES)

    xT = nc.declare_dram_parameter("xT", [D, M], F32, isOutput=False)
    xT_h = nc.declare_dram_parameter("xT_h", [D, M], F16, isOutput=False)
    css = nc.declare_dram_parameter("css", [2, P, M], F16, isOutput=False)
    wq_t = nc.declare_dram_parameter("wq_t", [NH, P, NKP, P], F16, isOutput=False)
    wk_t = nc.declare_dram_parameter("wk_t", [NH, P, NKP, P], F16, isOutput=False)
    wv_n = nc.declare_dram_parameter("wv_n", [D, QC], F16, isOutput=False)
    wo_t = nc.declare_dram_parameter("wo_t", [NH, P, NDT, P], F16, isOutput=False)
    wf_t = nc.declare_dram_parameter("wf_t", [NFT, P, NKP, P], F16, isOutput=False)
    wu_t = nc.declare_dram_parameter("wu_t", [NFT, P, NKP, P], F16, isOutput=False)
    wout_t = nc.declare_dram_parameter("wout_t", [NDT, P, NFT, P], F16, isOutput=False)
    y = nc.declare_dram_parameter("y", [D, M], F32, isOutput=True)

    with tile.TileContext(nc) as tc:
        _emit(nc, tc, xT, xT_h, css, wq_t, wk_t, wv_n, wo_t, wf_t, wu_t, wout_t, y)
    nc.compile()
    return nc


def _emit(nc, tc, xT, xT_h, css, wq_t, wk_t, wv_n, wo_t, wf_t, wu_t, wout_t, y):
    with ExitStack() as top:
        dram_pool = top.enter_context(tc.tile_pool(name="dram", bufs=1, space="DRAM"))
        const = top.enter_context(tc.tile_pool(name="const", bufs=1))

        cc_in = dram_pool.tile([D, M], F16)
        crows = (NDT // AR_CHUNKS) * P
        cc_out = [
            dram_pool.tile([crows, M], F16, addr_space="Shared", name=f"cc_out_{k}")
            for k in range(AR_CHUNKS)
        ]

        def xmid_rows(kp, cols=slice(None)):
            k, loc = divmod(kp, NDT // AR_CHUNKS)
            return cc_out[k][loc * P : (loc + 1) * P, cols]

        ones_h = const.tile([P, P], F16)
        nc.vector.memset(ones_h[:], 1.0)
        cc_sb = const.tile([P, M], F16)
        ss_sb = const.tile([P, M], F16)
        nc.sync.dma_start(out=cc_sb[:], in_=css[0])
        nc.sync.dma_start(out=ss_sb[:], in_=css[1])
        bcast1 = const.tile([P, M], F16)
        bcast2 = const.tile([P, M], F16)
        eps_sb = const.tile([P, 1], F32)
        nc.vector.memset(eps_sb[:], EPS)

        # ---- rms-norm statistics: bcast_out[:, t] = rsqrt(ms_t + eps) ----
        def norm_pass(src, bcast_out, tag):
            with ExitStack() as ph:
                sp = ph.enter_context(tc.tile_pool(name=f"norm_{tag}", bufs=1))
                pp = ph.enter_context(
                    tc.tile_pool(name=f"norm_ps_{tag}", bufs=1, space="PSUM")
                )
                ms_ps = pp.tile([P, M], F32, name=f"ms_ps_{tag}")
                for kp in range(NKP):
                    xt = sp.tile([P, M], F16, tag="xs", bufs=3, name=f"xs_{tag}_{kp}")
                    nc.sync.dma_start(out=xt[:], in_=src(kp))
                    sq = sp.tile([P, M], F16, tag="sq", bufs=3, name=f"sq_{tag}_{kp}")
                    if kp % 2 == 0:
                        nc.scalar.activation(sq[:], xt[:], AF.Square)
                    else:
                        nc.vector.tensor_mul(sq[:], xt[:], xt[:])
                    for ch in range(M // 512):
                        nc.tensor.matmul(
                            ms_ps[:, ch * 512 : (ch + 1) * 512],
                            ones_h[:],
                            sq[:, ch * 512 : (ch + 1) * 512],
                            start=(kp == 0),
                            stop=(kp == NKP - 1),
                        )
                lnt = sp.tile([P, M], F32, name=f"lnt_{tag}")
                nc.scalar.activation(
                    lnt[:], ms_ps[:], AF.Ln, bias=eps_sb[:], scale=1.0 / D
                )
                nc.scalar.activation(bcast_out[:], lnt[:], AF.Exp, scale=-0.5)

        norm_pass(lambda kp: xT_h[kp * P : (kp + 1) * P, :], bcast1, "1")

        # ------- per batch: qkv + rope + attention + o-proj partial -------
        for b in range(B):
            bs = slice(b * T, (b + 1) * T)
            with ExitStack() as bph:
                bp = bph.enter_context(tc.tile_pool(name=f"bat_{b}", bufs=1))
                qf, kf, v_sb = [], [], []
                with ExitStack() as ph:
                    xp = ph.enter_context(tc.tile_pool(name=f"xn_{b}", bufs=1))
                    sp = ph.enter_context(tc.tile_pool(name=f"qkv_{b}", bufs=1))
                    xn = []
                    for kp in range(NKP):
                        xt = sp.tile(
                            [P, T], F16, tag="xs", bufs=3, name=f"xq_{b}_{kp}"
                        )
                        nc.sync.dma_start(
                            out=xt[:], in_=xT_h[kp * P : (kp + 1) * P, bs]
                        )
                        xnk = xp.tile([P, T], F16, tag=f"xn{kp}", name=f"xn_{b}_{kp}")
                        nc.vector.tensor_mul(xnk[:], xt[:], bcast1[:, bs])
                        xn.append(xnk)

                    # q/k projections, rope fused into the eviction
                    with ExitStack() as qph:
                        qpp = qph.enter_context(
                            tc.tile_pool(name=f"qk_ps_{b}", bufs=1, space="PSUM")
                        )
                        for which, wsrc, dst in (("q", wq_t, qf), ("k", wk_t, kf)):
                            for m in range(NH):
                                wt = sp.tile(
                                    [P, NKP, P], F16, tag="wqk", bufs=3,
                                    name=f"w{which}_{b}_{m}",
                                )
                                nc.sync.dma_start(out=wt[:], in_=wsrc[m])
                                ps = qpp.tile(
                                    [P, T], F32, tag="qk_ps", bufs=2,
                                    name=f"ps{which}_{b}_{m}",
                                )
                                for ch in range(T // 512):
                                    cs = slice(ch * 512, (ch + 1) * 512)
                                    for kp in range(NKP):
                                        nc.tensor.matmul(
                                            ps[:, cs],
                                            wt[:, kp, :],
                                            xn[kp][:, cs],
                                            start=(kp == 0),
                                            stop=(kp == NKP - 1),
                                        )
                                main = sp.tile(
                                    [P, T], F16, tag="rmain", bufs=2,
                                    name=f"rm_{which}_{b}_{m}",
                                )
                                nc.vector.scalar_tensor_tensor(
                                    main[:], ps[:], 1.0, cc_sb[:, bs],
                                    ALU.mult, ALU.mult,
                                )
                                rot = sp.tile(
                                    [P, T], F16, tag="rrot", bufs=2,
                                    name=f"rr_{which}_{b}_{m}",
                                )
                                nc.vector.scalar_tensor_tensor(
                                    rot[:HALF], ps[HALF:], -1.0,
                                    ss_sb[:HALF, bs], ALU.mult, ALU.mult,
                                )
                                nc.vector.scalar_tensor_tensor(
                                    rot[HALF:], ps[:HALF], 1.0,
                                    ss_sb[HALF:, bs], ALU.mult, ALU.mult,
                                )
                                out = bp.tile(
                                    [P, T], F16, tag=f"{which}f{m}",
                                    name=f"{which}f_{b}_{m}",
                                )
                                nc.vector.tensor_add(out[:], main[:], rot[:])
                                dst.append(out)

                    # v projection, token-major: 8 concurrent accumulators
                    with ExitStack() as vph:
                        vpp = vph.enter_context(
                            tc.tile_pool(name=f"v_ps_{b}", bufs=1, space="PSUM")
                        )
                        ps_v = [
                            vpp.tile([P, QC], F32, tag=f"vps{st}",
                                     name=f"psv_{b}_{st}")
                            for st in range(NST)
                        ]
                        for kp in range(NKP):
                            wvk = sp.tile(
                                [P, QC], F16, tag="wv", bufs=3, name=f"wv_{b}_{kp}"
                            )
                            nc.sync.dma_start(
                                out=wvk[:], in_=wv_n[kp * P : (kp + 1) * P, :]
                            )
                            for st in range(NST):
                                nc.tensor.matmul(
                                    ps_v[st][:],
                                    xn[kp][:, st * P : (st + 1) * P],
                                    wvk[:],
                                    start=(kp == 0),
                                    stop=(kp == NKP - 1),
                                )
                        for st in range(NST):
                            vt = bp.tile(
                                [P, QC], F16, tag=f"v{st}", name=f"v_{b}_{st}"
                            )
                            nc.scalar.copy(vt[:], ps_v[st][:])
                            v_sb.append(vt)

                # attention per head
                attnf = []
                afp = bph.enter_context(tc.tile_pool(name=f"attnf_{b}", bufs=1))
                with ExitStack() as ah:
                    ap_ = ah.enter_context(tc.tile_pool(name=f"att_{b}", bufs=1))
                    app = ah.enter_context(
                        tc.tile_pool(name=f"att_ps_{b}", bufs=1, space="PSUM")
                    )
                    for h in range(NH):
                        den_ps = app.tile([P, T], F32, tag="den", name=f"den_{b}_{h}")
                        at_ps = app.tile([P, T], F32, tag="at", name=f"at_{b}_{h}")

                        def emit_lg(st):
                            lg_ps = app.tile(
                                [P, T], F32, tag="lg", bufs=2,
                                name=f"lg_{b}_{h}_{st}",
                            )
                            for ch in range(T // 512):
                                cs = slice(ch * 512, (ch + 1) * 512)
                                nc.tensor.matmul(
                                    lg_ps[:, cs],
                                    kf[h][:, st * P : (st + 1) * P],
                                    qf[h][:, cs],
                                    start=True,
                                    stop=True,
                                )
                            pr = ap_.tile(
                                [P, T], F16, tag="probs", bufs=4,
                                name=f"pr_{b}_{h}_{st}",
                            )
                            for ch in range(T // 512):
                                cs = slice(ch * 512, (ch + 1) * 512)
                                nc.scalar.activation(
                                    pr[:, cs], lg_ps[:, cs], AF.Exp
                                )
                            return pr

                        # software-pipelined: logits/exp of st+1 are emitted
                        # before den/pv of st so the PE has work during exp
                        prs = [None] * NST
                        prs[0] = emit_lg(0)
                        for st in range(NST):
                            if st + 1 < NST:
                                prs[st + 1] = emit_lg(st + 1)
                            pr = prs[st]
                            for ch in range(T // 512):
                                cs = slice(ch * 512, (ch + 1) * 512)
                                nc.tensor.matmul(
                                    den_ps[:, cs],
                                    ones_h[:],
                                    pr[:, cs],
                                    start=(st == 0),
                                    stop=(st == NST - 1),
                                )
                            for ch in range(T // 512):
                                cs = slice(ch * 512, (ch + 1) * 512)
                                nc.tensor.matmul(
                                    at_ps[:, cs],
                                    v_sb[st][:, h * H : (h + 1) * H],
                                    pr[:, cs],
                                    start=(st == 0),
                                    stop=(st == NST - 1),
                                )
                        af = afp.tile([P, T], F16, tag=f"af{h}", name=f"af_{b}_{h}")
                        for ch in range(T // 512):
                            cs = slice(ch * 512, (ch + 1) * 512)
                            rec = ap_.tile(
                                [P, 512], F32, tag="rec", bufs=4,
                                name=f"rec_{b}_{h}_{ch}",
                            )
                            nc.vector.reciprocal(rec[:], den_ps[:, cs])
                            nc.vector.scalar_tensor_tensor(
                                af[:, cs], at_ps[:, cs], 1.0, rec[:],
                                ALU.mult, ALU.mult,
                            )
                        attnf.append(af)

                # o-projection partial for this batch (columns = batch)
                with ExitStack() as ph:
                    sp = ph.enter_context(tc.tile_pool(name=f"op_{b}", bufs=1))
                    pp = ph.enter_context(
                        tc.tile_pool(name=f"op_ps_{b}", bufs=1, space="PSUM")
                    )
                    wo_sb = []
                    for h in range(NH):
                        wt = sp.tile(
                            [P, NDT, P], F16, tag=f"wo{h}", name=f"wo_{b}_{h}"
                        )
                        nc.sync.dma_start(out=wt[:], in_=wo_t[h])
                        wo_sb.append(wt)
                    for dt in range(NDT):
                        ps = pp.tile(
                            [P, T], F32, tag="o_ps", bufs=2, name=f"pso_{b}_{dt}"
                        )
                        for ch in range(T // 512):
                            cs = slice(ch * 512, (ch + 1) * 512)
                            for h in range(NH):
                                nc.tensor.matmul(
                                    ps[:, cs],
                                    wo_sb[h][:, dt, :],
                                    attnf[h][:, cs],
                                    start=(h == 0),
                                    stop=(h == NH - 1),
                                )
                        xt = sp.tile(
                            [P, T], F32, tag="xs3", bufs=3, name=f"xo_{b}_{dt}"
                        )
                        nc.sync.dma_start(
                            out=xt[:], in_=xT[dt * P : (dt + 1) * P, bs]
                        )
                        osb = sp.tile(
                            [P, T], F16, tag="osb", bufs=3, name=f"osb_{b}_{dt}"
                        )
                        nc.vector.scalar_tensor_tensor(
                            osb[:], xt[:], 1.0 / N_CORES, ps[:],
                            ALU.mult, ALU.add,
                        )
                        nc.sync.dma_start(
                            out=cc_in[dt * P : (dt + 1) * P, bs], in_=osb[:]
                        )
                        if b == B - 1 and (dt + 1) % (NDT // AR_CHUNKS) == 0:
                            k = (dt + 1) // (NDT // AR_CHUNKS) - 1
                            rows = slice(
                                (dt + 1 - NDT // AR_CHUNKS) * P, (dt + 1) * P
                            )
                            nc.gpsimd.collective_compute(
                                "AllReduce",
                                ALU.add,
                                replica_groups=[list(range(N_CORES))],
                                ins=[cc_in[rows, :]],
                                outs=[cc_out[k][:, :]],
                            )

        # ---------------- norm 2 ----------------
        norm_pass(xmid_rows, bcast2, "2")

        # ---------------- SwiGLU MLP (per token-half) ----------------
        for hb in range(B):
            bs = slice(hb * T, (hb + 1) * T)
            with ExitStack() as bph:
                bp = bph.enter_context(tc.tile_pool(name=f"mlpb_{hb}", bufs=1))
                hsb = []
                with ExitStack() as ph:
                    xp = ph.enter_context(tc.tile_pool(name=f"xm_{hb}", bufs=1))
                    sp = ph.enter_context(tc.tile_pool(name=f"mlp_{hb}", bufs=1))
                    pp = ph.enter_context(
                        tc.tile_pool(name=f"mlp_ps_{hb}", bufs=1, space="PSUM")
                    )
                    # fp16 AllReduced stream used directly as matmul rhs
                    xmh = []
                    for kp in range(NKP):
                        xk = xp.tile([P, T], F16, tag=f"xm{kp}", name=f"xmh_{hb}_{kp}")
                        nc.sync.dma_start(out=xk[:], in_=xmid_rows(kp, bs))
                        xmh.append(xk)

                    ffs = []
                    for m in range(NFT):
                        for which, wsrc in (("f", wf_t), ("u", wu_t)):
                            wt = sp.tile(
                                [P, NKP, P], F16, tag="wffu", bufs=3,
                                name=f"w{which}_{hb}_{m}",
                            )
                            nc.sync.dma_start(out=wt[:], in_=wsrc[m])
                            ps = pp.tile(
                                [P, T], F32, tag=f"ps_{which}", bufs=2,
                                name=f"ps{which}_{hb}_{m}",
                            )
                            for ch in range(T // 512):
                                cs = slice(ch * 512, (ch + 1) * 512)
                                for kp in range(NKP):
                                    nc.tensor.matmul(
                                        ps[:, cs],
                                        wt[:, kp, :],
                                        xmh[kp][:, cs],
                                        start=(kp == 0),
                                        stop=(kp == NKP - 1),
                                    )
                            # fold the norm2 scale into the eviction
                            nt = sp.tile(
                                [P, T], F16, tag=f"nrm_{which}", bufs=3,
                                name=f"nt{which}_{hb}_{m}",
                            )
                            nc.vector.scalar_tensor_tensor(
                                nt[:], ps[:], 1.0, bcast2[:, bs],
                                ALU.mult, ALU.mult,
                            )
                            if which == "f":
                                ft = sp.tile(
                                    [P, T], F16, tag="ffs", bufs=3,
                                    name=f"ff_{hb}_{m}",
                                )
                                nc.scalar.activation(ft[:], nt[:], AF.Silu)
                                ffs.append(ft)
                            else:
                                ht = bp.tile(
                                    [P, T], F16, tag=f"h{m}", name=f"h_{hb}_{m}"
                                )
                                nc.vector.tensor_mul(ht[:], nt[:], ffs[m][:])
                                hsb.append(ht)

                # w_out projection + residual, partial output
                with ExitStack() as ph:
                    sp = ph.enter_context(tc.tile_pool(name=f"wo2_{hb}", bufs=1))
                    pp = ph.enter_context(
                        tc.tile_pool(name=f"wo2_ps_{hb}", bufs=1, space="PSUM")
                    )
                    for dt in range(NDT):
                        wt = sp.tile(
                            [P, NFT, P], F16, tag="wot", bufs=3,
                            name=f"wot_{hb}_{dt}",
                        )
                        nc.sync.dma_start(out=wt[:], in_=wout_t[dt])
                        ps = pp.tile(
                            [P, T], F32, tag="ps_o2", bufs=2, name=f"pso2_{hb}_{dt}"
                        )
                        for ch in range(T // 512):
                            cs = slice(ch * 512, (ch + 1) * 512)
                            for m in range(NFT):
                                nc.tensor.matmul(
                                    ps[:, cs],
                                    wt[:, m, :],
                                    hsb[m][:, cs],
                                    start=(m == 0),
                                    stop=(m == NFT - 1),
                                )
                        xm = sp.tile(
                            [P, T], F16, tag="xs4", bufs=3, name=f"xm2_{hb}_{dt}"
                        )
                        nc.sync.dma_start(out=xm[:], in_=xmid_rows(dt, bs))
                        ysb = sp.tile(
                            [P, T], F32, tag="ysb", bufs=3, name=f"ysb_{hb}_{dt}"
                        )
                        nc.vector.scalar_tensor_tensor(
                            ysb[:], xm[:], 1.0 / N_CORES, ps[:], ALU.mult, ALU.add
                        )
                        nc.sync.dma_start(
                            out=y[dt * P : (dt + 1) * P, bs], in_=ysb[:]
                        )


_NC_CACHE = {}


def _get_nc():
    if "nc" not in _NC_CACHE:
        _NC_CACHE["nc"] = _build()
    return _NC_CACHE["nc"]


def _host_prep(x, sin, cos, attn_norm_w, ff_norm_w, wq, wk, wv, wo, w_ff, w_up, w_out):
    f16 = np.float16
    x2 = np.asarray(x, np.float32).reshape(M, D)
    xT = np.ascontiguousarray(x2.T)

    sinT = np.asarray(sin, np.float32).reshape(M, HALF).T
    cosT = np.asarray(cos, np.float32).reshape(M, HALF).T
    cc = np.concatenate([cosT, cosT], axis=0)
    ss = np.concatenate([sinT, sinT], axis=0)
    css = np.stack([cc, ss]).astype(f16)

    anw = np.asarray(attn_norm_w, np.float32)[:, None]
    fnw = np.asarray(ff_norm_w, np.float32)[:, None]
    wqn = (anw * np.asarray(wq, np.float32)) * (H ** -0.5)
    wkn = anw * np.asarray(wk, np.float32)
    wvn = anw * np.asarray(wv, np.float32)
    wfn = fnw * np.asarray(w_ff, np.float32)
    wun = fnw * np.asarray(w_up, np.float32)
    wo = np.asarray(wo, np.float32)
    w_out = np.asarray(w_out, np.float32)

    def mtile(w):
        # [K, F] -> [F/P, P, K/P, P] with [m, p, kp, j] = w[kp*P+p, m*P+j]
        K, F = w.shape
        return np.ascontiguousarray(
            w.reshape(K // P, P, F // P, P).transpose(2, 1, 0, 3)
        )

    in_maps = []
    for c in range(N_CORES):
        qs = slice(c * QC, (c + 1) * QC)
        fs = slice(c * FC, (c + 1) * FC)
        in_maps.append(
            {
                "xT": xT,
                "xT_h": xT.astype(f16),
                "css": css,
                "wq_t": mtile(wqn[:, qs]).astype(f16),
                "wk_t": mtile(wkn[:, qs]).astype(f16),
                "wv_n": wvn[:, qs].astype(f16),
                # [h, p, dt, j] = wo[c*QC + h*P + p, dt*P + j]
                "wo_t": np.ascontiguousarray(
                    wo[qs, :].reshape(NH, P, NDT, P)
                ).astype(f16),
                "wf_t": mtile(wfn[:, fs]).astype(f16),
                "wu_t": mtile(wun[:, fs]).astype(f16),
                "wout_t": mtile(w_out[fs, :]).astype(f16),
            }
        )
    return in_maps


def kernel(**inputs) -> np.ndarray:
    nc = _get_nc()
    in_maps = _host_prep(**inputs)
    res = run_bass_kernel_spmd(
        nc, in_maps, core_ids=list(range(N_CORES)), trace=False
    )
    acc = res.results[0]["y"].astype(np.float64)
    for c in range(1, N_CORES):
        acc += res.results[c]["y"]
    return np.ascontiguousarray(acc.T).astype(np.float32).reshape(B, T, D)



# revision 4
# speedup vs baseline: 1.2262x; 1.2262x over previous
"""TP-8 Trainium2 Bass kernel for a LLaDA/Llama transformer block.

Shapes (hardcoded): x [2, 1024, 4096], 32 heads x 128 head_dim,
FF=12288, non-causal attention, RMSNorm + RoPE + SwiGLU.

Sharding (per sharding_hint): tensor-parallel over the 8 cores —
q/k/v/ff sharded on the output-feature axis (4 heads / 1536 ff dims per
core), wo/w_out sharded on the contraction axis.  One bf16 on-device
AllReduce per batch restores the residual stream after attention; the
final projection partials are summed on the host.

v2 schedule (vs the fp16 v1 baseline at 2.58 ms):
 - per-batch pipeline: qkv(b0) attn(b0) oproj(b0)+AR(b0) qkv... so each
   batch's AllReduce overlaps the other batch's compute / the MLP.
 - bf16 operands (FWL-eligible weight loads), consecutive matmuls share
   a stationary via ch-inner loop order.
 - x->xn shares one SBUF tile ring; xmid tiles are shared between the
   norm2 stats, the ff/up moving operands and the final residual.
 - norm scale rs = exp(-0.5*ln(mean_sq + eps)); norm weights folded into
   the adjacent weight matrices on the host; 1/sqrt(head_dim) in wq.
 - cross-partition sums (sum over D, softmax denominator) use an
   all-ones stationary operand; softmax needs no max subtraction.
"""

from contextlib import ExitStack

import numpy as np
import ml_dtypes

import concourse.mybir as mybir
import concourse.tile as tile
from concourse import bacc
from concourse.bass_utils import run_bass_kernel_spmd

F32 = mybir.dt.float32
DT = mybir.dt.bfloat16
NPDT = ml_dtypes.bfloat16
AF = mybir.ActivationFunctionType
ALU = mybir.AluOpType

N_CORES = 8
P = 128
B, T, D, FF = 2, 1024, 4096, 12288
M = B * T            # 2048 tokens
H = 128              # head dim
HALF = 64
QC = D // N_CORES    # 512 per-core q/k/v features (4 heads)
NH = QC // H         # 4 heads per core
FC = FF // N_CORES   # 1536 per-core ff features
NKP = D // P         # 32 K-tiles over D
NFT = FC // P        # 12 M-tiles over per-core FF
NDT = D // P         # 32 D-tiles
NST = T // P         # 8 sequence tiles per batch
EPS = 1e-05


def _build():
    nc = bacc.Bacc("TRN2", target_bir_lowering=False, num_devices=N_CORES)

    xT_h = nc.declare_dram_parameter("xT_h", [D, M], DT, isOutput=False)
    css = nc.declare_dram_parameter("css", [2, P, M], DT, isOutput=False)
    wq_t = nc.declare_dram_parameter("wq_t", [NH, P, NKP, P], DT, isOutput=False)
    wk_t = nc.declare_dram_parameter("wk_t", [NH, P, NKP, P], DT, isOutput=False)
    wv_n = nc.declare_dram_parameter("wv_n", [D, QC], DT, isOutput=False)
    wo_t = nc.declare_dram_parameter("wo_t", [NH, P, NDT, P], DT, isOutput=False)
    wf_t = nc.declare_dram_parameter("wf_t", [NFT, P, NKP, P], DT, isOutput=False)
    wu_t = nc.declare_dram_parameter("wu_t", [NFT, P, NKP, P], DT, isOutput=False)
    wout_t = nc.declare_dram_parameter("wout_t", [NDT, P, NFT, P], DT, isOutput=False)
    y = nc.declare_dram_parameter("y", [D, M], F32, isOutput=True)

    with tile.TileContext(nc) as tc:
        _emit(nc, tc, xT_h, css, wq_t, wk_t, wv_n, wo_t, wf_t, wu_t, wout_t, y)
    nc.compile()
    return nc


def _emit(nc, tc, xT_h, css, wq_t, wk_t, wv_n, wo_t, wf_t, wu_t, wout_t, y):
    with ExitStack() as top:
        dram_pool = top.enter_context(tc.tile_pool(name="dram", bufs=1, space="DRAM"))
        const = top.enter_context(tc.tile_pool(name="const", bufs=1))

        cc_in = [dram_pool.tile([D, T], DT, name=f"cc_in_{b}") for b in range(B)]
        cc_out = [
            dram_pool.tile([D, T], DT, addr_space="Shared", name=f"cc_out_{b}")
            for b in range(B)
        ]

        ones_h = const.tile([P, P], DT)
        nc.vector.memset(ones_h[:], 1.0)
        cc_sb = const.tile([P, M], DT)
        ss_sb = const.tile([P, M], DT)
        nc.sync.dma_start(out=cc_sb[:], in_=css[0])
        nc.sync.dma_start(out=ss_sb[:], in_=css[1])
        eps_sb = const.tile([P, 1], F32)
        nc.vector.memset(eps_sb[:], EPS)

        # ---------- attention + o-proj per batch; AR fires per batch ----------
        for b in range(B):
            bs = slice(b * T, (b + 1) * T)
            with ExitStack() as bph:
                bp = bph.enter_context(tc.tile_pool(name=f"bat_{b}", bufs=1))
                qf, kf, v_sb = [], [], []
                with ExitStack() as ph:
                    sp = ph.enter_context(tc.tile_pool(name=f"qkv_{b}", bufs=1))
                    # ---- rms-norm stats for this batch ----
                    xs = []
                    for kp in range(NKP):
                        xt = sp.tile([P, T], DT, tag="xs", bufs=34, name=f"x_{b}_{kp}")
                        nc.sync.dma_start(out=xt[:], in_=xT_h[kp * P : (kp + 1) * P, bs])
                        xs.append(xt)
                    bcast1 = sp.tile([P, T], DT, name=f"bc1_{b}")
                    with ExitStack() as sph:
                        spp = sph.enter_context(
                            tc.tile_pool(name=f"st_ps_{b}", bufs=1, space="PSUM")
                        )
                        ms_ps = spp.tile([P, T], F32, name=f"ms_ps_{b}")
                        for kp in range(NKP):
                            sq = sp.tile([P, T], DT, tag="sq", bufs=3, name=f"sq_{b}_{kp}")
                            if kp % 2 == 0:
                                nc.scalar.activation(sq[:], xs[kp][:], AF.Square)
                            else:
                                nc.vector.tensor_mul(sq[:], xs[kp][:], xs[kp][:])
                            for ch in range(T // 512):
                                cs = slice(ch * 512, (ch + 1) * 512)
                                nc.tensor.matmul(
                                    ms_ps[:, cs], ones_h[:], sq[:, cs],
                                    start=(kp == 0), stop=(kp == NKP - 1),
                                )
                        lnt = sp.tile([P, T], F32, name=f"lnt_{b}")
                        nc.scalar.activation(
                            lnt[:], ms_ps[:], AF.Ln, bias=eps_sb[:], scale=1.0 / D
                        )
                        nc.scalar.activation(bcast1[:], lnt[:], AF.Exp, scale=-0.5)
                    # xn tiles recycle the x ring slots
                    xn = []
                    for kp in range(NKP):
                        xnk = sp.tile([P, T], DT, tag="xs", bufs=34, name=f"xn_{b}_{kp}")
                        nc.vector.tensor_mul(xnk[:], xs[kp][:], bcast1[:])
                        xn.append(xnk)

                    # ---- q/k projections, rope fused into the eviction ----
                    with ExitStack() as qph:
                        qpp = qph.enter_context(
                            tc.tile_pool(name=f"qk_ps_{b}", bufs=1, space="PSUM")
                        )
                        for which, wsrc, dst in (("q", wq_t, qf), ("k", wk_t, kf)):
                            for m in range(NH):
                                wt = sp.tile(
                                    [P, NKP, P], DT, tag="wqk", bufs=3,
                                    name=f"w{which}_{b}_{m}",
                                )
                                nc.sync.dma_start(out=wt[:], in_=wsrc[m])
                                ps = qpp.tile(
                                    [P, T], F32, tag="qk_ps", bufs=2,
                                    name=f"ps{which}_{b}_{m}",
                                )
                                for kp in range(NKP):
                                    for ch in range(T // 512):
                                        cs = slice(ch * 512, (ch + 1) * 512)
                                        nc.tensor.matmul(
                                            ps[:, cs], wt[:, kp, :], xn[kp][:, cs],
                                            start=(kp == 0), stop=(kp == NKP - 1),
                                        )
                                main = sp.tile(
                                    [P, T], DT, tag="rmain", bufs=2,
                                    name=f"rm_{which}_{b}_{m}",
                                )
                                nc.vector.scalar_tensor_tensor(
                                    main[:], ps[:], 1.0, cc_sb[:, bs],
                                    ALU.mult, ALU.mult,
                                )
                                rot = sp.tile(
                                    [P, T], DT, tag="rrot", bufs=2,
                                    name=f"rr_{which}_{b}_{m}",
                                )
                                nc.vector.scalar_tensor_tensor(
                                    rot[:HALF], ps[HALF:], -1.0,
                                    ss_sb[:HALF, bs], ALU.mult, ALU.mult,
                                )
                                nc.vector.scalar_tensor_tensor(
                                    rot[HALF:], ps[:HALF], 1.0,
                                    ss_sb[HALF:, bs], ALU.mult, ALU.mult,
                                )
                                out = bp.tile(
                                    [P, T], DT, tag=f"{which}f{m}",
                                    name=f"{which}f_{b}_{m}",
                                )
                                nc.vector.tensor_add(out[:], main[:], rot[:])
                                dst.append(out)

                    # ---- v projection, token-major: 8 concurrent accumulators
                    with ExitStack() as vph:
                        vpp = vph.enter_context(
                            tc.tile_pool(name=f"v_ps_{b}", bufs=1, space="PSUM")
                        )
                        ps_v = [
                            vpp.tile([P, QC], F32, tag=f"vps{st}", name=f"psv_{b}_{st}")
                            for st in range(NST)
                        ]
                        for kp in range(NKP):
                            wvk = sp.tile(
                                [P, QC], DT, tag="wv", bufs=3, name=f"wv_{b}_{kp}"
                            )
                            nc.sync.dma_start(
                                out=wvk[:], in_=wv_n[kp * P : (kp + 1) * P, :]
                            )
                            for st in range(NST):
                                nc.tensor.matmul(
                                    ps_v[st][:],
                                    xn[kp][:, st * P : (st + 1) * P],
                                    wvk[:],
                                    start=(kp == 0), stop=(kp == NKP - 1),
                                )
                        for st in range(NST):
                            vt = bp.tile([P, QC], DT, tag=f"v{st}", name=f"v_{b}_{st}")
                            nc.scalar.copy(vt[:], ps_v[st][:])
                            v_sb.append(vt)

                # ---- attention per head ----
                attnf = []
                afp = bph.enter_context(tc.tile_pool(name=f"attnf_{b}", bufs=1))
                with ExitStack() as ah:
                    ap_ = ah.enter_context(tc.tile_pool(name=f"att_{b}", bufs=1))
                    app = ah.enter_context(
                        tc.tile_pool(name=f"att_ps_{b}", bufs=1, space="PSUM")
                    )
                    for h in range(NH):
                        den_ps = app.tile([P, T], F32, tag="den", name=f"den_{b}_{h}")
                        at_ps = app.tile([P, T], F32, tag="at", name=f"at_{b}_{h}")

                        def emit_lg(st):
                            lg_ps = app.tile(
                                [P, T], F32, tag="lg", bufs=2, name=f"lg_{b}_{h}_{st}"
                            )
                            for ch in range(T // 512):
                                cs = slice(ch * 512, (ch + 1) * 512)
                                nc.tensor.matmul(
                                    lg_ps[:, cs],
                                    kf[h][:, st * P : (st + 1) * P],
                                    qf[h][:, cs],
                                    start=True, stop=True,
                                )
                            pr = ap_.tile(
                                [P, T], DT, tag="probs", bufs=4,
                                name=f"pr_{b}_{h}_{st}",
                            )
                            for ch in range(T // 512):
                                cs = slice(ch * 512, (ch + 1) * 512)
                                nc.scalar.activation(pr[:, cs], lg_ps[:, cs], AF.Exp)
                            return pr

                        # software-pipelined: logits/exp of st+1 are emitted
                        # before den/pv of st so the PE has work during exp
                        prs = [None] * NST
                        prs[0] = emit_lg(0)
                        for st in range(NST):
                            if st + 1 < NST:
                                prs[st + 1] = emit_lg(st + 1)
                            pr = prs[st]
                            for ch in range(T // 512):
                                cs = slice(ch * 512, (ch + 1) * 512)
                                nc.tensor.matmul(
                                    den_ps[:, cs], ones_h[:], pr[:, cs],
                                    start=(st == 0), stop=(st == NST - 1),
                                )
                            for ch in range(T // 512):
                                cs = slice(ch * 512, (ch + 1) * 512)
                                nc.tensor.matmul(
                                    at_ps[:, cs],
                                    v_sb[st][:, h * H : (h + 1) * H],
                                    pr[:, cs],
                                    start=(st == 0), stop=(st == NST - 1),
                                )
                        af = afp.tile([P, T], DT, tag=f"af{h}", name=f"af_{b}_{h}")
                        for ch in range(T // 512):
                            cs = slice(ch * 512, (ch + 1) * 512)
                            rec = ap_.tile(
                                [P, 512], F32, tag="rec", bufs=4,
                                name=f"rec_{b}_{h}_{ch}",
                            )
                            nc.vector.reciprocal(rec[:], den_ps[:, cs])
                            nc.vector.scalar_tensor_tensor(
                                af[:, cs], at_ps[:, cs], 1.0, rec[:],
                                ALU.mult, ALU.mult,
                            )
                        attnf.append(af)

                # ---- o-projection partial for this batch ----
                with ExitStack() as ph:
                    sp = ph.enter_context(tc.tile_pool(name=f"op_{b}", bufs=1))
                    pp = ph.enter_context(
                        tc.tile_pool(name=f"op_ps_{b}", bufs=1, space="PSUM")
                    )
                    wo_sb = []
                    for h in range(NH):
                        wt = sp.tile([P, NDT, P], DT, tag=f"wo{h}", name=f"wo_{b}_{h}")
                        nc.sync.dma_start(out=wt[:], in_=wo_t[h])
                        wo_sb.append(wt)
                    for dt in range(NDT):
                        ps = pp.tile([P, T], F32, tag="o_ps", bufs=2, name=f"pso_{b}_{dt}")
                        for h in range(NH):
                            for ch in range(T // 512):
                                cs = slice(ch * 512, (ch + 1) * 512)
                                nc.tensor.matmul(
                                    ps[:, cs], wo_sb[h][:, dt, :], attnf[h][:, cs],
                                    start=(h == 0), stop=(h == NH - 1),
                                )
                        xt = sp.tile([P, T], DT, tag="xs3", bufs=3, name=f"xo_{b}_{dt}")
                        nc.sync.dma_start(
                            out=xt[:], in_=xT_h[dt * P : (dt + 1) * P, bs]
                        )
                        osb = sp.tile([P, T], DT, tag="osb", bufs=3, name=f"osb_{b}_{dt}")
                        nc.vector.scalar_tensor_tensor(
                            osb[:], xt[:], 1.0 / N_CORES, ps[:], ALU.mult, ALU.add
                        )
                        nc.sync.dma_start(
                            out=cc_in[b][dt * P : (dt + 1) * P, :], in_=osb[:]
                        )
                    nc.gpsimd.collective_compute(
                        "AllReduce",
                        ALU.add,
                        replica_groups=[list(range(N_CORES))],
                        ins=[cc_in[b][:, :]],
                        outs=[cc_out[b][:, :]],
                    )

        # ---------------- SwiGLU MLP per batch-half ----------------
        for hb in range(B):
            bs = slice(hb * T, (hb + 1) * T)
            with ExitStack() as bph:
                bp = bph.enter_context(tc.tile_pool(name=f"mlpb_{hb}", bufs=1))
                hsb = []
                xmh = []
                with ExitStack() as ph:
                    xp = ph.enter_context(tc.tile_pool(name=f"xm_{hb}", bufs=1))
                    sp = ph.enter_context(tc.tile_pool(name=f"mlp_{hb}", bufs=1))
                    # xmid tiles: stats + ff/up moving operand + residual
                    for kp in range(NKP):
                        xk = xp.tile([P, T], DT, tag=f"xm{kp}", name=f"xmh_{hb}_{kp}")
                        nc.sync.dma_start(
                            out=xk[:], in_=cc_out[hb][kp * P : (kp + 1) * P, :]
                        )
                        xmh.append(xk)
                    bcast2 = sp.tile([P, T], DT, name=f"bc2_{hb}")
                    with ExitStack() as sph:
                        spp = sph.enter_context(
                            tc.tile_pool(name=f"st2_ps_{hb}", bufs=1, space="PSUM")
                        )
                        ms2 = spp.tile([P, T], F32, name=f"ms2_{hb}")
                        for kp in range(NKP):
                            sq = sp.tile(
                                [P, T], DT, tag="sq2", bufs=3, name=f"sq2_{hb}_{kp}"
                            )
                            if kp % 2 == 0:
                                nc.scalar.activation(sq[:], xmh[kp][:], AF.Square)
                            else:
                                nc.vector.tensor_mul(sq[:], xmh[kp][:], xmh[kp][:])
                            for ch in range(T // 512):
                                cs = slice(ch * 512, (ch + 1) * 512)
                                nc.tensor.matmul(
                                    ms2[:, cs], ones_h[:], sq[:, cs],
                                    start=(kp == 0), stop=(kp == NKP - 1),
                                )
                        lnt2 = sp.tile([P, T], F32, name=f"lnt2_{hb}")
                        nc.scalar.activation(
                            lnt2[:], ms2[:], AF.Ln, bias=eps_sb[:], scale=1.0 / D
                        )
                        nc.scalar.activation(bcast2[:], lnt2[:], AF.Exp, scale=-0.5)

                    ffs = []
                    with ExitStack() as fph:
                        fpp = fph.enter_context(
                            tc.tile_pool(name=f"ffu_ps_{hb}", bufs=1, space="PSUM")
                        )
                        for m in range(NFT):
                            for which, wsrc in (("f", wf_t), ("u", wu_t)):
                                wt = sp.tile(
                                    [P, NKP, P], DT, tag="wffu", bufs=3,
                                    name=f"w{which}_{hb}_{m}",
                                )
                                nc.sync.dma_start(out=wt[:], in_=wsrc[m])
                                ps = fpp.tile(
                                    [P, T], F32, tag=f"ps_{which}", bufs=2,
                                    name=f"ps{which}_{hb}_{m}",
                                )
                                for kp in range(NKP):
                                    for ch in range(T // 512):
                                        cs = slice(ch * 512, (ch + 1) * 512)
                                        nc.tensor.matmul(
                                            ps[:, cs], wt[:, kp, :], xmh[kp][:, cs],
                                            start=(kp == 0), stop=(kp == NKP - 1),
                                        )
                                # fold the norm2 scale into the eviction
                                nt = sp.tile(
                                    [P, T], DT, tag=f"nrm_{which}", bufs=3,
                                    name=f"nt{which}_{hb}_{m}",
                                )
                                nc.vector.scalar_tensor_tensor(
                                    nt[:], ps[:], 1.0, bcast2[:], ALU.mult, ALU.mult
                                )
                                if which == "f":
                                    ft = sp.tile(
                                        [P, T], DT, tag="ffs", bufs=3,
                                        name=f"ff_{hb}_{m}",
                                    )
                                    nc.scalar.activation(ft[:], nt[:], AF.Silu)
                                    ffs.append(ft)
                                else:
                                    ht = bp.tile(
                                        [P, T], DT, tag=f"h{m}", name=f"h_{hb}_{m}"
                                    )
                                    nc.vector.tensor_mul(ht[:], nt[:], ffs[m][:])
                                    hsb.append(ht)

                    # w_out projection + residual, partial output
                    with ExitStack() as oph:
                        opp = oph.enter_context(
                            tc.tile_pool(name=f"wo2_ps_{hb}", bufs=1, space="PSUM")
                        )
                        for dt in range(NDT):
                            wt = sp.tile(
                                [P, NFT, P], DT, tag="wot", bufs=3,
                                name=f"wot_{hb}_{dt}",
                            )
                            nc.sync.dma_start(out=wt[:], in_=wout_t[dt])
                            ps = opp.tile(
                                [P, T], F32, tag="ps_o2", bufs=2,
                                name=f"pso2_{hb}_{dt}",
                            )
                            for m in range(NFT):
                                for ch in range(T // 512):
                                    cs = slice(ch * 512, (ch + 1) * 512)
                                    nc.tensor.matmul(
                                        ps[:, cs], wt[:, m, :], hsb[m][:, cs],
                                        start=(m == 0), stop=(m == NFT - 1),
                                    )
                            ysb = sp.tile(
                                [P, T], F32, tag="ysb", bufs=3, name=f"ysb_{hb}_{dt}"
                            )
                            nc.vector.scalar_tensor_tensor(
                                ysb[:], xmh[dt][:], 1.0 / N_CORES, ps[:],
                                ALU.mult, ALU.add,
                            )
                            nc.sync.dma_start(
                                out=y[dt * P : (dt + 1) * P, bs], in_=ysb[:]
                            )


_NC_CACHE = {}


def _get_nc():
    if "nc" not in _NC_CACHE:
        _NC_CACHE["nc"] = _build()
    return _NC_CACHE["nc"]


def _host_prep(x, sin, cos, attn_norm_w, ff_norm_w, wq, wk, wv, wo, w_ff, w_up, w_out):
    x2 = np.asarray(x, np.float32).reshape(M, D)
    xT = np.ascontiguousarray(x2.T)

    sinT = np.asarray(sin, np.float32).reshape(M, HALF).T
    cosT = np.asarray(cos, np.float32).reshape(M, HALF).T
    cc = np.concatenate([cosT, cosT], axis=0)
    ss = np.concatenate([sinT, sinT], axis=0)
    css = np.stack([cc, ss]).astype(NPDT)

    anw = np.asarray(attn_norm_w, np.float32)[:, None]
    fnw = np.asarray(ff_norm_w, np.float32)[:, None]
    wqn = (anw * np.asarray(wq, np.float32)) * (H ** -0.5)
    wkn = anw * np.asarray(wk, np.float32)
    wvn = anw * np.asarray(wv, np.float32)
    wfn = fnw * np.asarray(w_ff, np.float32)
    wun = fnw * np.asarray(w_up, np.float32)
    wo = np.asarray(wo, np.float32)
    w_out = np.asarray(w_out, np.float32)

    def mtile(w):
        # [K, F] -> [F/P, P, K/P, P] with [m, p, kp, j] = w[kp*P+p, m*P+j]
        K, F = w.shape
        return np.ascontiguousarray(
            w.reshape(K // P, P, F // P, P).transpose(2, 1, 0, 3)
        )

    in_maps = []
    for c in range(N_CORES):
        qs = slice(c * QC, (c + 1) * QC)
        fs = slice(c * FC, (c + 1) * FC)
        in_maps.append(
            {
                "xT_h": xT.astype(NPDT),
                "css": css,
                "wq_t": mtile(wqn[:, qs]).astype(NPDT),
                "wk_t": mtile(wkn[:, qs]).astype(NPDT),
                "wv_n": wvn[:, qs].astype(NPDT),
                # [h, p, dt, j] = wo[c*QC + h*P + p, dt*P + j]
                "wo_t": np.ascontiguousarray(
                    wo[qs, :].reshape(NH, P, NDT, P)
                ).astype(NPDT),
                "wf_t": mtile(wfn[:, fs]).astype(NPDT),
                "wu_t": mtile(wun[:, fs]).astype(NPDT),
                "wout_t": mtile(w_out[fs, :]).astype(NPDT),
            }
        )
    return in_maps


def kernel(**inputs) -> np.ndarray:
    nc = _get_nc()
    in_maps = _host_prep(**inputs)
    res = run_bass_kernel_spmd(
        nc, in_maps, core_ids=list(range(N_CORES)), trace=False
    )
    acc = res.results[0]["y"].astype(np.float64)
    for c in range(1, N_CORES):
        acc += res.results[c]["y"]
    return np.ascontiguousarray(acc.T).astype(np.float32).reshape(B, T, D)
